# revision 1
# baseline (speedup 1.0000x reference)
"""Trainium2 Bass kernel for LogHarmonicLowering.

out[b, k*C + c, j, t] = wv0[k,j] * x[b, c, j+d_k, t] + wv1[k,j] * x[b, c, j+d_k+1, t]

with zero padding past the frequency range. The bilinear shift per k has a
constant integer part d_k plus per-(k,j) float32 weights wv0/wv1 precomputed
on host with the exact float32 arithmetic of the reference's grid method.

Distribution: data-parallel over batch — 8 cores, one batch element each.

Per-core scheme: partition dim = frequency. Compute-engine SBUF operands must
start at partition 0/32/64/96, so partition-shifted operand reads are illegal.
Instead:
  - the +1 row shift is materialized at load time: X1 = x shifted down one
    frequency row (DMA reads have no partition-offset restriction),
  - the integer shift d_k is applied at store time via the store DMA's
    SBUF-side partition offset.
Everything on ACT/DVE is then a full-tile partition-aligned op in INPUT row
space i (output row j = i - d):
  ACT:  tmp[i] = wv1[k, i-d] * X1[i]         (Copy activation, per-row scale)
  DVE:  Z[i]   = (X[i] * wv0[k, i-d]) + tmp[i]   (scalar_tensor_tensor)
  store out[., i-d, .] = Z[i]  for i in [d, 256)
k = 0 (shift 0) is an exact copy: input tiles are stored straight back.
Trailing d zero rows come from a persistent zeroed tile.
"""

import functools

import numpy as np

import concourse.bass as bass
import concourse.mybir as mybir
from concourse.bass_utils import run_bass_kernel_spmd
from concourse.tile import TileContext

FK = 5
ANCHOR = 1
OUT_LOG = 12.0
IN_LOG = 1.0
RADIX = 2.0

B, C, F, T = 8, 32, 256, 512
N_CORES = 8


def _host_weights(Fr):
    """Per-(k, j) bilinear weights, float32 ops matching the jax reference."""
    np_shift = (np.arange(FK) + 1) / ANCHOR
    ls = OUT_LOG * np.log(IN_LOG * np_shift) / np.log(RADIX)
    ls -= ls[ANCHOR - 1]
    ls32 = ls.astype(np.float32)
    shift_px = ls32 * np.float32(Fr / (Fr - 1))
    y = np.arange(Fr, dtype=np.float32)[None, :] + shift_px[:, None]
    y0f = np.floor(y)
    w1 = y - y0f
    w0 = np.float32(1.0) - w1
    y0 = y0f.astype(np.int32)
    y1 = y0 + 1
    v0 = ((y0 >= 0) & (y0 < Fr)).astype(np.float32)
    v1 = ((y1 >= 0) & (y1 < Fr)).astype(np.float32)
    wv0 = w0 * v0
    wv1 = w1 * v1
    d = y0[:, 0]
    # the integer shift is constant along j (fractional parts never round
    # across an integer boundary in f32 for these shifts)
    assert (y0 == d[:, None] + np.arange(Fr, dtype=np.int32)[None, :]).all()
    return wv0, wv1, d


def build_nc(C=C, Fr=F, T=T, G=4, NBUF=3, TBUF=2, ZBUF=3):
    """Raw-bass per-core program: x[C,Fr,T] -> out[FK*C,Fr,T].

    Hand-scheduled pipeline (this toolchain's walrus allows only ONE sync
    wait per instruction, so Tile's attached multi-waits cannot compile;
    every wait here is its own wait_ge instruction).
      SP : all DMAs (one HWDGE ring -> count-based completion waits are safe)
      ACT: tmp = wv1 * X1 (Copy activation, per-partition scale)
      DVE: Z = (X * wv0) + tmp (scalar_tensor_tensor), zeros memset
    """
    from contextlib import ExitStack

    wv0, wv1, dks = _host_weights(Fr)
    H = Fr // 2
    nG = C // G
    f32 = mybir.dt.float32
    dmax = int(dks.max())

    ncols = 2 * (FK - 1) * 2

    def col(a, ki, t):
        return (a * (FK - 1) + ki) * 2 + t

    wvtab = np.zeros((H, ncols), np.float32)
    idx = np.arange(H)
    for ki in range(FK - 1):
        k = ki + 1
        d = int(dks[k])
        for a, wv in ((0, wv0), (1, wv1)):
            av = np.where(idx >= d, wv[k, np.maximum(idx - d, 0)], np.float32(0))
            wvtab[:, col(a, ki, 0)] = av
            wvtab[:, col(a, ki, 1)] = wv[k, idx + H - d]

    Copy = mybir.ActivationFunctionType.Copy
    mult = mybir.AluOpType.mult
    add = mybir.AluOpType.add

    nc = bass.Bass(trn_type="TRN2")
    x_h = nc.dram_tensor("x", [C, Fr, T], f32, kind="ExternalInput")
    out_h = nc.dram_tensor("out", [FK * C, Fr, T], f32, kind="ExternalOutput")
    wv_h = nc.inline_tensor(wvtab, name="wvtab")

    def dram(ap):
        return ap.rearrange("c f t -> f c t")

    # op-count bookkeeping (1-based)
    def act_after(g, k):          # s_act value once tmpB(g,k) done
        return 8 * g + 2 * k + 2

    def dve_after(g, k):          # s_dve value once ZB(g,k) done (memset=#1)
        return 8 * g + 2 * k + 3

    def store_idx(g, k=None, which=0):
        # cumulative SP store count: per group 2 k0 + 3 per k
        base = 14 * g
        if k is None:
            return base + 2
        return base + 2 + 3 * k + which  # which: 1=ZA, 2=ZB, 3=zeros

    load_after = {}  # g -> cumulative load-DMA count once group g loaded

    with ExitStack() as ctx:
        sb = lambda shape, name: ctx.enter_context(
            nc.sbuf_tensor(name, shape, f32))
        wvt = sb([H, ncols], "wvt")
        zeros = sb([dmax, G, T], "zeros")
        XA = [sb([H, G, T], f"XA{s}") for s in range(NBUF)]
        XB = [sb([H, G, T], f"XB{s}") for s in range(NBUF)]
        X1A = [sb([H, G, T], f"X1A{s}") for s in range(NBUF)]
        X1B = [sb([H, G, T], f"X1B{s}") for s in range(NBUF)]
        tA = [sb([H, G, T], f"tA{s}") for s in range(TBUF)]
        tB = [sb([H, G, T], f"tB{s}") for s in range(TBUF)]
        ZA = [sb([H, G, T], f"ZA{s}") for s in range(ZBUF)]
        ZB = [sb([H, G, T], f"ZB{s}") for s in range(ZBUF)]
        sem = lambda name: ctx.enter_context(nc.semaphore(name))
        s_wv = sem("s_wv")
        s_ld = [sem(f"s_ld{s}") for s in range(NBUF)]
        s_xst = [sem(f"s_xst{s}") for s in range(NBUF)]
        s_zst = [sem(f"s_zst{s}") for s in range(ZBUF)]
        s_zr = sem("s_zr")
        s_act = sem("s_act")
        s_dve = sem("s_dve")
        block = ctx.enter_context(nc.Block())

        class W:  # monotone wait elision per engine
            def __init__(self, e):
                self.e, self.seen = e, {}
            def __call__(self, sem_, v):
                if v > self.seen.get(id(sem_), 0):
                    self.e.wait_ge(sem_, v)
                    self.seen[id(sem_)] = v

        nzr = [0]

        @block.sync
        def _(e):
            w = W(e)
            e.dma_start(out=wvt[:, :], in_=wv_h[:, :]).then_inc(s_wv, 16)

            def issue_loads(g):
                s = g % NBUF
                u = g // NBUF
                w(s_ld[s], 80 * u)          # own-sem order for detector
                xg = x_h[g * G:(g + 1) * G, :, :]
                e.dma_start(out=XA[s][:, :, :], in_=dram(xg[:, 0:H, :])).then_inc(s_ld[s], 16)
                e.dma_start(out=XB[s][:, :, :], in_=dram(xg[:, H:Fr, :])).then_inc(s_ld[s], 16)
                e.dma_start(out=X1A[s][:, :, :], in_=dram(xg[:, 1:H + 1, :])).then_inc(s_ld[s], 16)
                e.dma_start(out=X1B[s][0:H - 1, :, :], in_=dram(xg[:, H + 1:Fr, :])).then_inc(s_ld[s], 16)
                e.dma_start(out=X1B[s][H - 1:H, :, :], in_=dram(xg[:, Fr - 1:Fr, :])).then_inc(s_ld[s], 16)

            for g in range(min(NBUF, nG)):
                issue_loads(g)
            for g in range(nG):
                s = g % NBUF
                u = g // NBUF
                og0 = out_h[g * G:(g + 1) * G, :, :]
                w(s_ld[s], 80 * (u + 1))
                w(s_xst[s], 32 * u)
                e.dma_start(out=dram(og0[:, 0:H, :]), in_=XA[s][:, :, :]).then_inc(s_xst[s], 16)
                e.dma_start(out=dram(og0[:, H:Fr, :]), in_=XB[s][:, :, :]).then_inc(s_xst[s], 16)
                for k in range(FK - 1):
                    d = int(dks[k + 1])
                    i = 4 * g + k
                    z = i % ZBUF
                    uz = i // ZBUF
                    og = out_h[(k + 1) * C + g * G:(k + 1) * C + (g + 1) * G, :, :]
                    w(s_zst[z], 32 * uz)
                    w(s_dve, dve_after(g, k) - 1)   # ZA ready
                    e.dma_start(out=dram(og[:, 0:H - d, :]), in_=ZA[z][d:H, :, :]).then_inc(s_zst[z], 16)
                    w(s_dve, dve_after(g, k))       # ZB ready
                    e.dma_start(out=dram(og[:, H - d:Fr - d, :]), in_=ZB[z][:, :, :]).then_inc(s_zst[z], 16)
                    w(s_zr, 16 * max(0, nzr[0] - 8))
                    e.dma_start(out=dram(og[:, Fr - d:Fr, :]), in_=zeros[0:d, :, :]).then_inc(s_zr, 16)
                    nzr[0] += 1
                gn = g + NBUF
                if gn < nG:
                    # recycle slot: ACT/DVE consumed group g, k0 stores landed
                    w(s_act, act_after(g, FK - 2))
                    w(s_dve, dve_after(g, FK - 2))
                    w(s_xst[s], 32 * (u + 1))
                    issue_loads(gn)
            # drain every DMA sem before program end
            w(s_zr, 16 * nzr[0])
            for z in range(ZBUF):
                uses = sum(1 for i in range(4 * nG) if i % ZBUF == z)
                w(s_zst[z], 32 * uses)
            for s in range(NBUF):
                uses = sum(1 for g in range(nG) if g % NBUF == s)
                w(s_xst[s], 32 * uses)

        @block.scalar
        def _(e):
            w = W(e)
            w(s_wv, 16)
            for g in range(nG):
                s = g % NBUF
                u = g // NBUF
                for k in range(FK - 1):
                    i = 4 * g + k
                    t = i % TBUF
                    w(s_ld[s], 80 * (u + 1))
                    if i >= TBUF:  # tmp slot recycle: reader stt of pair i-TBUF
                        g2, k2 = divmod(i - TBUF, 4)
                        w(s_dve, dve_after(g2, k2))
                    e.activation(tA[t][:, :, :], X1A[s][:, :, :], Copy,
                                 scale=wvt[:, col(1, k, 0):col(1, k, 0) + 1]
                                 ).then_inc(s_act, 1)
                    e.activation(tB[t][:, :, :], X1B[s][:, :, :], Copy,
                                 scale=wvt[:, col(1, k, 1):col(1, k, 1) + 1]
                                 ).then_inc(s_act, 1)

        @block.vector
        def _(e):
            w = W(e)
            e.memset(zeros[:, :, :], 0.0).then_inc(s_dve, 1)
            w(s_wv, 16)
            for g in range(nG):
                s = g % NBUF
                u = g // NBUF
                for k in range(FK - 1):
                    i = 4 * g + k
                    t = i % TBUF
                    z = i % ZBUF
                    uz = i // ZBUF
                    w(s_act, act_after(g, k))
                    w(s_ld[s], 80 * (u + 1))
                    w(s_zst[z], 32 * uz)   # Z slot recycle: prior stores done
                    e.scalar_tensor_tensor(
                        ZA[z][:, :, :], XA[s][:, :, :],
                        wvt[:, col(0, k, 0):col(0, k, 0) + 1],
                        tA[t][:, :, :], mult, add).then_inc(s_dve, 1)
                    e.scalar_tensor_tensor(
                        ZB[z][:, :, :], XB[s][:, :, :],
                        wvt[:, col(0, k, 1):col(0, k, 1) + 1],
                        tB[t][:, :, :], mult, add).then_inc(s_dve, 1)
    return nc


@functools.lru_cache(maxsize=1)
def _get_nc():
    return build_nc()


def _run(x, trace=False):
    in_maps = [{"x": np.ascontiguousarray(x[b])} for b in range(B)]
    res = run_bass_kernel_spmd(_get_nc(), in_maps, core_ids=list(range(N_CORES)),
                               trace=trace)
    out = np.stack([r["out"] for r in res.results], axis=0)
    return out, res


def kernel(x):
    x = np.asarray(x)
    assert x.shape == (B, C, F, T), x.shape
    out, _ = _run(x)
    return out



# revision 9
# speedup vs baseline: 100.7894x; 100.7894x over previous
"""Trainium2 Bass kernel for LogHarmonicLowering.

out[b, k*C + c, j, t] = wv0[k,j] * x[b, c, j+d_k, t] + wv1[k,j] * x[b, c, j+d_k+1, t]

with zero padding past the frequency range. The bilinear shift per k has a
constant integer part d_k plus per-(k,j) float32 weights wv0/wv1 precomputed
on host with the exact float32 arithmetic of the reference's grid method.

Distribution: data-parallel over batch — 8 cores, one batch element each.

Per-core scheme: partition dim = frequency row within a 128-row half; the
half index h lives on the free axis, so each channel-group tile is
X[p, h, c, t] = x[c, h*128+p, t] with shape [128, 2, G, T]. X1 is the same
tile pre-shifted down one frequency row (loaded directly from DRAM — DMA
reads have no partition-offset restriction). Per (group, k):

  ACT:  Z[:, h] = wv1[k, j(h,p)] * X1[:, h]          (Copy activation,
                                                      per-partition scale)
  DVE:  Z[:, h] = (X[:, h] * wv0[k, j(h,p)]) + Z[:, h]  (scalar_tensor_tensor,
                                                         in place on Z)
  DVE issues the two store DMAs for Z right after computing it; the
  integer shift d_k is applied via the store DMA's SBUF-side partition
  offset. k = 0 (shift 0) is an exact copy: X tiles are stored straight
  back by the sync engine. Trailing d_k zero rows come from a persistent
  zeroed tile.

Hand-scheduled BSP pipeline (one sync wait per instruction; all semaphores
are global monotone counters):
  SP : weight-table + X/X1 loads (prefetch NBUF groups deep), k0 stores,
       zero-row stores, final drain
  ACT: Z = wv1 * X1
  DVE: zeros memset, Z = X*wv0 + Z, Z store issue
"""

import functools

import numpy as np

import concourse.bass as bass
import concourse.mybir as mybir
from concourse.bass_utils import run_bass_kernel_spmd

FK = 5
ANCHOR = 1
OUT_LOG = 12.0
IN_LOG = 1.0
RADIX = 2.0

B, C, F, T = 8, 32, 256, 512
N_CORES = 8


def _host_weights(Fr):
    """Per-(k, j) bilinear weights, float32 ops matching the jax reference."""
    np_shift = (np.arange(FK) + 1) / ANCHOR
    ls = OUT_LOG * np.log(IN_LOG * np_shift) / np.log(RADIX)
    ls -= ls[ANCHOR - 1]
    ls32 = ls.astype(np.float32)
    shift_px = ls32 * np.float32(Fr / (Fr - 1))
    y = np.arange(Fr, dtype=np.float32)[None, :] + shift_px[:, None]
    y0f = np.floor(y)
    w1 = y - y0f
    w0 = np.float32(1.0) - w1
    y0 = y0f.astype(np.int32)
    y1 = y0 + 1
    v0 = ((y0 >= 0) & (y0 < Fr)).astype(np.float32)
    v1 = ((y1 >= 0) & (y1 < Fr)).astype(np.float32)
    wv0 = w0 * v0
    wv1 = w1 * v1
    d = y0[:, 0]
    # the integer shift is constant along j (fractional parts never round
    # across an integer boundary in f32 for these shifts)
    assert (y0 == d[:, None] + np.arange(Fr, dtype=np.int32)[None, :]).all()
    return wv0, wv1, d


def build_nc(C=C, Fr=F, T=T, G=4, NBUF=3, ZBUF=3, reps=1):
    """Raw-bass per-core program: x[C,Fr,T] -> out[FK*C,Fr,T]."""
    from contextlib import ExitStack

    wv0, wv1, dks = _host_weights(Fr)
    H = Fr // 2
    assert H == 128
    nG = C // G
    nK = FK - 1
    f32 = mybir.dt.float32
    dmax = int(dks.max())

    # weight table: 16 columns, one per (a in {wv0, wv1}, ki in 0..3, h in 0..1)
    ncols = 2 * nK * 2

    def col(a, ki, h):
        return (a * nK + ki) * 2 + h

    wvtab = np.zeros((H, ncols), np.float32)
    p = np.arange(H)
    for ki in range(nK):
        k = ki + 1
        d = int(dks[k])
        for a, wv in ((0, wv0), (1, wv1)):
            # h=0: output row j = p - d (invalid rows weighted 0, never stored)
            wvtab[:, col(a, ki, 0)] = np.where(
                p >= d, wv[k, np.maximum(p - d, 0)], np.float32(0))
            # h=1: output row j = H + p - d (always in range)
            wvtab[:, col(a, ki, 1)] = wv[k, p + H - d]

    Copy = mybir.ActivationFunctionType.Copy
    mult = mybir.AluOpType.mult
    add = mybir.AluOpType.add

    nc = bass.Bass(trn_type="TRN2")
    x_h = nc.dram_tensor("x", [C, Fr, T], f32, kind="ExternalInput")
    out_h = nc.dram_tensor("out", [FK * C, Fr, T], f32, kind="ExternalOutput")
    wv_h = nc.inline_tensor(wvtab, name="wvtab")

    def dram2(ap):
        # [G, 2H, T] dram slice -> [p, h, c, t]
        return ap.rearrange("c (h p) t -> p c h t", h=2)

    def dram1(ap):
        # [G, rows, T] dram slice -> [p, 1, c, t]
        return ap.rearrange("c (o p) t -> p c o t", o=1)

    # cumulative sem targets (each DMA completion +16, each compute +1).
    # DMA-completion semaphores are per-buffer-slot: the race detector
    # requires the issuing engine to have waited on a sem (covering all
    # prior increments) before starting a new burst of increments on it.
    LD_PER_G = 4 * 16          # X, X1h0, X1h1main, X1clamp

    def act_after(g, ki, n=2):
        return 8 * g + 2 * ki + n

    with ExitStack() as ctx:
        sb = lambda shape, name: ctx.enter_context(
            nc.sbuf_tensor(name, shape, f32))
        wvt = sb([H, ncols], "wvt")
        zeros = sb([dmax, G, 1, T], "zeros")
        X = [sb([H, G, 2, T], f"X{s}") for s in range(NBUF)]
        X1 = [sb([H, G, 2, T], f"X1{s}") for s in range(NBUF)]
        Z = [sb([H, G, 2, T], f"Z{s}") for s in range(ZBUF)]
        sem = lambda name: ctx.enter_context(nc.semaphore(name))
        s_wv = sem("s_wv")
        s_ld = [sem(f"s_ld{s}") for s in range(NBUF)]
        s_act = sem("s_act")
        s_dve = sem("s_dve")
        s_stk = [sem(f"s_stk{s}") for s in range(2)]   # k0 copy stores
        s_st = [sem(f"s_st{z}") for z in range(ZBUF)]  # Z stores
        s_z = sem("s_z")       # memset + zero-row stores
        block = ctx.enter_context(nc.Block())

        class W:  # monotone wait elision per engine
            def __init__(self, e):
                self.e, self.seen = e, {}

            def __call__(self, sem_, v):
                if v > self.seen.get(id(sem_), 0):
                    self.e.wait_ge(sem_, v)
                    self.seen[id(sem_)] = v

        @block.sync
        def _(e):
            w = W(e)
            e.dma_start(out=wvt[:, :], in_=wv_h[:, :]).then_inc(s_wv, 16)

            def issue_loads(gg):
                g = gg % nG
                s = gg % NBUF
                u = gg // NBUF
                w(s_ld[s], LD_PER_G * u)   # own-sem burst order for detector
                xg = x_h[g * G:(g + 1) * G, :, :]
                e.dma_start(out=X[s][:, :, :, :],
                            in_=dram2(xg)).then_inc(s_ld[s], 16)
                e.dma_start(out=X1[s][:, :, 0:1, :],
                            in_=dram1(xg[:, 1:H + 1, :])).then_inc(s_ld[s], 16)
                e.dma_start(out=X1[s][0:H - 1, :, 1:2, :],
                            in_=dram1(xg[:, H + 1:Fr, :])).then_inc(s_ld[s], 16)
                e.dma_start(out=X1[s][H - 1:H, :, 1:2, :],
                            in_=dram1(xg[:, Fr - 1:Fr, :])).then_inc(s_ld[s], 16)

            nGT = nG * reps
            for gg in range(min(NBUF, nGT)):
                issue_loads(gg)
            for gg in range(nGT):
                g = gg % nG
                s = gg % NBUF
                # k0 straight copy (needs group gg loads complete)
                og0 = out_h[g * G:(g + 1) * G, :, :]
                w(s_ld[s], LD_PER_G * (gg // NBUF + 1))
                w(s_stk[gg % 2], 16 * (gg // 2))   # own-sem burst order
                e.dma_start(out=dram2(og0), in_=X[s][:, :, :, :]
                            ).then_inc(s_stk[gg % 2], 16)
                # zero-row tails for each k (independent of compute)
                w(s_z, 1 + 4 * 16 * gg)            # own-sem burst order
                for ki in range(nK):
                    k = ki + 1
                    d = int(dks[k])
                    og = out_h[k * C + g * G:k * C + (g + 1) * G, :, :]
                    e.dma_start(out=dram1(og[:, Fr - d:Fr, :]),
                                in_=zeros[0:d, :, :, :]).then_inc(s_z, 16)
                gn = gg + NBUF
                if gn < nGT:
                    # recycle slot: group gg fully consumed
                    w(s_act, act_after(gg, nK - 1))    # X1 read done
                    w(s_dve, act_after(gg, nK - 1))    # X read done (stt)
                    w(s_stk[gg % 2], 16 * (gg // 2 + 1))  # k0 store done
                    issue_loads(gn)
            # drain every DMA sem before program end
            for z in range(ZBUF):
                uses = sum(1 for i in range(nK * nGT) if i % ZBUF == z)
                w(s_st[z], 32 * uses)
            for par in range(2):
                uses = sum(1 for gg in range(nGT) if gg % 2 == par)
                w(s_stk[par], 16 * uses)
            w(s_z, 1 + 4 * 16 * nGT)
            for s in range(NBUF):
                uses = sum(1 for gg in range(nGT) if gg % NBUF == s)
                w(s_ld[s], LD_PER_G * uses)

        @block.scalar
        def _(e):
            w = W(e)
            w(s_wv, 16)
            for gg in range(nG * reps):
                s = gg % NBUF
                w(s_ld[s], LD_PER_G * (gg // NBUF + 1))
                for ki in range(nK):
                    i = 4 * gg + ki
                    z = i % ZBUF
                    if i >= ZBUF:   # Z slot recycle: prior stores done
                        w(s_st[z], 32 * (i // ZBUF))
                    e.activation(Z[z][:, :, 0:1, :], X1[s][:, :, 0:1, :], Copy,
                                 scale=wvt[:, col(1, ki, 0):col(1, ki, 0) + 1]
                                 ).then_inc(s_act, 1)
                    e.activation(Z[z][:, :, 1:2, :], X1[s][:, :, 1:2, :], Copy,
                                 scale=wvt[:, col(1, ki, 1):col(1, ki, 1) + 1]
                                 ).then_inc(s_act, 1)

        @block.vector
        def _(e):
            w = W(e)
            e.memset(zeros[:, :, :, :], 0.0).then_inc(s_z, 1)
            w(s_wv, 16)
            for gg in range(nG * reps):
                s = gg % NBUF
                for ki in range(nK):
                    i = 4 * gg + ki
                    z = i % ZBUF
                    w(s_act, act_after(gg, ki, 1))
                    e.scalar_tensor_tensor(
                        Z[z][:, :, 0:1, :], X[s][:, :, 0:1, :],
                        wvt[:, col(0, ki, 0):col(0, ki, 0) + 1],
                        Z[z][:, :, 0:1, :], mult, add).then_inc(s_dve, 1)
                    w(s_act, act_after(gg, ki, 2))
                    e.scalar_tensor_tensor(
                        Z[z][:, :, 1:2, :], X[s][:, :, 1:2, :],
                        wvt[:, col(0, ki, 1):col(0, ki, 1) + 1],
                        Z[z][:, :, 1:2, :], mult, add).then_inc(s_dve, 1)

        @block.gpsimd
        def _(e):
            w = W(e)
            for gg in range(nG * reps):
                g = gg % nG
                for ki in range(nK):
                    k = ki + 1
                    d = int(dks[k])
                    i = 4 * gg + ki
                    z = i % ZBUF
                    og = out_h[k * C + g * G:k * C + (g + 1) * G, :, :]
                    w(s_st[z], 32 * (i // ZBUF))   # own-sem burst order
                    w(s_dve, 2 * i + 1)
                    e.dma_start(out=dram1(og[:, 0:H - d, :]),
                                in_=Z[z][d:H, :, 0:1, :]).then_inc(s_st[z], 16)
                    w(s_dve, 2 * i + 2)
                    e.dma_start(out=dram1(og[:, H - d:Fr - d, :]),
                                in_=Z[z][:, :, 1:2, :]).then_inc(s_st[z], 16)
    return nc


@functools.lru_cache(maxsize=1)
def _get_nc():
    return build_nc()


def _run(x, trace=False):
    in_maps = [{"x": np.ascontiguousarray(x[b])} for b in range(B)]
    res = run_bass_kernel_spmd(_get_nc(), in_maps, core_ids=list(range(N_CORES)),
                               trace=trace)
    out = np.stack([r["out"] for r in res.results], axis=0)
    return out, res


def kernel(x):
    x = np.asarray(x)
    assert x.shape == (B, C, F, T), x.shape
    out, _ = _run(x)
    return out


# revision 18
# speedup vs baseline: 161.4591x; 1.6019x over previous
"""Trainium2 Bass kernel for LogHarmonicLowering.

out[b, k*C + c, j, t] = wv0[k,j] * x[b, c, j+d_k, t] + wv1[k,j] * x[b, c, j+d_k+1, t]

with zero padding past the frequency range. The bilinear shift per k has a
constant integer part d_k plus per-(k,j) float32 weights wv0/wv1 precomputed
on host with the exact float32 arithmetic of the reference's grid method.

Distribution: data-parallel over batch — 8 cores, one batch element each.

Per-core scheme: partition dim = frequency row within a 128-row half; the
half index h lives on the free axis, so each channel-group tile is
X[p, c, h, t] = x[c, h*128+p, t] with shape [128, G, 2, T]. X1 is the same
tile shifted down one frequency row. Per (group, k):

  ACT:  Z[:, :, h] = wv1[k, j(h,p)] * X1[:, :, h]       (Copy activation,
                                                         per-partition scale)
  DVE:  Z[:, :, h] = (X[:, :, h] * wv0[k, j(h,p)]) + Z[:, :, h]
                                         (scalar_tensor_tensor, in place)

The integer shift d_k is applied via the store DMA's SBUF-side partition
offset (DMAs have no partition-alignment restriction; compute operands
must start at partition 0/32/64/96, so a partition-shifted operand read is
illegal). k = 0 (shift 0) is an exact copy: X tiles are stored straight
back. Trailing d_k zero rows come from a persistent zeroed tile.

Two ways to materialize X1:
  pe_x1=False: load it from DRAM again (re-reads x, ~17MB extra traffic)
  pe_x1=True:  PE computes it into PSUM with exact 0/1 f32 shift matrices
               (X1A = S@XA + E@XB accumulated, X1B = S@XB; S = subdiagonal,
               E picks XB row 0 into row 127); ACT then reads PSUM.
               This removes the duplicate HBM read entirely.

Hand-scheduled BSP pipeline (one sync wait per instruction). All DMA
completion semaphores are per-buffer-slot and every issuing engine waits
on its own sem before starting a new burst of increments on it (the race
detector requires burst ordering; completions across bursts are otherwise
unordered).

Engines: SP loads/k0 copies/zero stores + drain, PE the X1 shift (pe_x1),
ACT the wv1 product, DVE the fused multiply-add, gpsimd the Z stores.

`reps` repeats the whole pipeline inside one program (buffer-slot
rotation continues across repeats) — used by test.py to measure per-
execution device time differentially; the program is idempotent.
"""

import functools

import numpy as np

import concourse.bass as bass
import concourse.mybir as mybir
from concourse.bass_utils import run_bass_kernel_spmd

FK = 5
ANCHOR = 1
OUT_LOG = 12.0
IN_LOG = 1.0
RADIX = 2.0

B, C, F, T = 8, 32, 256, 512
N_CORES = 8


def _host_weights(Fr):
    """Per-(k, j) bilinear weights, float32 ops matching the jax reference."""
    np_shift = (np.arange(FK) + 1) / ANCHOR
    ls = OUT_LOG * np.log(IN_LOG * np_shift) / np.log(RADIX)
    ls -= ls[ANCHOR - 1]
    ls32 = ls.astype(np.float32)
    shift_px = ls32 * np.float32(Fr / (Fr - 1))
    y = np.arange(Fr, dtype=np.float32)[None, :] + shift_px[:, None]
    y0f = np.floor(y)
    w1 = y - y0f
    w0 = np.float32(1.0) - w1
    y0 = y0f.astype(np.int32)
    y1 = y0 + 1
    v0 = ((y0 >= 0) & (y0 < Fr)).astype(np.float32)
    v1 = ((y1 >= 0) & (y1 < Fr)).astype(np.float32)
    wv0 = w0 * v0
    wv1 = w1 * v1
    d = y0[:, 0]
    # the integer shift is constant along j (fractional parts never round
    # across an integer boundary in f32 for these shifts)
    assert (y0 == d[:, None] + np.arange(Fr, dtype=np.int32)[None, :]).all()
    return wv0, wv1, d


def build_nc(C=C, Fr=F, T=T, G=2, NBUF=4, ZBUF=6, reps=1, pe_x1=True):
    """Raw-bass per-core program: x[C,Fr,T] -> out[FK*C,Fr,T]."""
    from contextlib import ExitStack

    wv0, wv1, dks = _host_weights(Fr)
    H = Fr // 2
    assert H == 128
    nG = C // G
    nGT = nG * reps
    nK = FK - 1
    f32 = mybir.dt.float32
    dmax = int(dks.max())

    # weight table: 16 columns, one per (a in {wv0, wv1}, ki in 0..3, h in 0..1)
    ncols = 2 * nK * 2

    def col(a, ki, h):
        return (a * nK + ki) * 2 + h

    wvtab = np.zeros((H, ncols), np.float32)
    p = np.arange(H)
    for ki in range(nK):
        k = ki + 1
        d = int(dks[k])
        for a, wv in ((0, wv0), (1, wv1)):
            # h=0: output row j = p - d (invalid rows weighted 0, never stored)
            wvtab[:, col(a, ki, 0)] = np.where(
                p >= d, wv[k, np.maximum(p - d, 0)], np.float32(0))
            # h=1: output row j = H + p - d (always in range)
            wvtab[:, col(a, ki, 1)] = wv[k, p + H - d]

    # PE shift matrices (lhsT layout: [K=i, M=p]): X1[p] = x[p+1]
    se = np.zeros((H, 2 * H), np.float32)
    se[np.arange(1, H), np.arange(H - 1)] = 1.0          # S: i = p+1
    se[0, H + 127] = 1.0                                 # E: row127 += XB[0]

    Copy = mybir.ActivationFunctionType.Copy
    mult = mybir.AluOpType.mult
    add = mybir.AluOpType.add

    nc = bass.Bass(trn_type="TRN2")
    x_h = nc.dram_tensor("x", [C, Fr, T], f32, kind="ExternalInput")
    out_h = nc.dram_tensor("out", [FK * C, Fr, T], f32, kind="ExternalOutput")
    wv_h = nc.inline_tensor(wvtab, name="wvtab")
    se_h = nc.inline_tensor(se, name="setab") if pe_x1 else None

    def dram2(ap):
        # [G, 2H, T] dram slice -> [p, c, h, t]
        return ap.rearrange("c (h p) t -> p c h t", h=2)

    def dram1(ap):
        # [G, rows, T] dram slice -> [p, c, 1, t]
        return ap.rearrange("c (o p) t -> p c o t", o=1)

    LD_PER_G = (1 if pe_x1 else 4) * 16
    wv_target = 32 if pe_x1 else 16

    def act_after(g, ki=nK - 1, n=2):
        return 8 * g + 2 * ki + n

    with ExitStack() as ctx:
        sb = lambda shape, name: ctx.enter_context(
            nc.sbuf_tensor(name, shape, f32))
        wvt = sb([H, ncols], "wvt")
        zeros = sb([dmax, G, 1, T], "zeros")
        X = [sb([H, G, 2, T], f"X{s}") for s in range(NBUF)]
        if pe_x1:
            set_ = sb([H, 2 * H], "set")
            # two full-group PSUM slots (4 banks each at G=2), rotating on
            # group parity so PE runs a whole group ahead of ACT
            ps = [ctx.enter_context(
                nc.psum_tensor(f"ps{b}", [H, G, 2, T], f32)) for b in range(2)]
            X1 = None
        else:
            X1 = [sb([H, G, 2, T], f"X1{s}") for s in range(NBUF)]
        Z = [sb([H, G, 2, T], f"Z{s}") for s in range(ZBUF)]
        sem = lambda name: ctx.enter_context(nc.semaphore(name))
        s_wv = sem("s_wv")
        s_ld = [sem(f"s_ld{s}") for s in range(NBUF)]
        s_pe = sem("s_pe")
        s_act = sem("s_act")
        s_dve = sem("s_dve")
        s_stk = [sem(f"s_stk{s}") for s in range(2)]   # k0 copy stores
        s_st = [sem(f"s_st{z}") for z in range(ZBUF)]  # Z stores
        s_z = sem("s_z")       # memset + zero-row stores
        block = ctx.enter_context(nc.Block())

        class W:  # monotone wait elision per engine
            def __init__(self, e):
                self.e, self.seen = e, {}

            def __call__(self, sem_, v):
                if v > self.seen.get(id(sem_), 0):
                    self.e.wait_ge(sem_, v)
                    self.seen[id(sem_)] = v

        @block.sync
        def _(e):
            w = W(e)
            e.dma_start(out=wvt[:, :], in_=wv_h[:, :]).then_inc(s_wv, 16)
            if pe_x1:
                e.dma_start(out=set_[:, :], in_=se_h[:, :]).then_inc(s_wv, 16)

            def issue_loads(gg):
                g = gg % nG
                s = gg % NBUF
                u = gg // NBUF
                w(s_ld[s], LD_PER_G * u)   # own-sem burst order for detector
                xg = x_h[g * G:(g + 1) * G, :, :]
                e.dma_start(out=X[s][:, :, :, :],
                            in_=dram2(xg)).then_inc(s_ld[s], 16)
                if not pe_x1:
                    e.dma_start(out=X1[s][:, :, 0:1, :],
                                in_=dram1(xg[:, 1:H + 1, :])
                                ).then_inc(s_ld[s], 16)
                    e.dma_start(out=X1[s][0:H - 1, :, 1:2, :],
                                in_=dram1(xg[:, H + 1:Fr, :])
                                ).then_inc(s_ld[s], 16)
                    e.dma_start(out=X1[s][H - 1:H, :, 1:2, :],
                                in_=dram1(xg[:, Fr - 1:Fr, :])
                                ).then_inc(s_ld[s], 16)

            for gg in range(min(NBUF, nGT)):
                issue_loads(gg)
            for gg in range(nGT):
                g = gg % nG
                s = gg % NBUF
                # k0 straight copy (needs group gg loads complete)
                og0 = out_h[g * G:(g + 1) * G, :, :]
                w(s_ld[s], LD_PER_G * (gg // NBUF + 1))
                w(s_stk[gg % 2], 16 * (gg // 2))   # own-sem burst order
                e.dma_start(out=dram2(og0), in_=X[s][:, :, :, :]
                            ).then_inc(s_stk[gg % 2], 16)
                # zero-row tails for each k (independent of compute)
                w(s_z, 1 + 4 * 16 * gg)            # own-sem burst order
                for ki in range(nK):
                    k = ki + 1
                    d = int(dks[k])
                    og = out_h[k * C + g * G:k * C + (g + 1) * G, :, :]
                    e.dma_start(out=dram1(og[:, Fr - d:Fr, :]),
                                in_=zeros[0:d, :, :, :]).then_inc(s_z, 16)
                gn = gg + NBUF
                if gn < nGT:
                    # recycle slot: group gg fully consumed
                    if pe_x1:
                        w(s_pe, 3 * G * (gg + 1))      # PE rhs reads done
                    else:
                        w(s_act, act_after(gg))        # X1 reads done
                    w(s_dve, act_after(gg))            # X reads done (stt)
                    w(s_stk[gg % 2], 16 * (gg // 2 + 1))  # k0 store done
                    issue_loads(gn)
            # drain every DMA sem before program end
            for z in range(ZBUF):
                uses = sum(1 for i in range(nK * nGT) if i % ZBUF == z)
                w(s_st[z], 32 * uses)
            for par in range(2):
                uses = sum(1 for gg in range(nGT) if gg % 2 == par)
                w(s_stk[par], 16 * uses)
            w(s_z, 1 + 4 * 16 * nGT)
            for s in range(NBUF):
                uses = sum(1 for gg in range(nGT) if gg % NBUF == s)
                w(s_ld[s], LD_PER_G * uses)

        if pe_x1:
            # per group (slot b = gg%2): for each c the same-bank group
            # [S@XA_c (start), E@XB_c (accum+stop)] -> ps[b] h0 bank, then
            # the standalone [S@XB_c] -> ps[b] h1 bank. PE runs one full
            # group ahead of ACT on the other slot.
            @block.tensor
            def _(e):
                w = W(e)
                w(s_wv, wv_target)
                S_ap = set_[:, 0:H]
                E_ap = set_[:, H:2 * H]
                for gg in range(nGT):
                    s = gg % NBUF
                    b = gg % 2
                    w(s_ld[s], LD_PER_G * (gg // NBUF + 1))
                    if gg >= 2:   # ps slot recycle: ACT of gg-2 done
                        w(s_act, 8 * (gg - 1))
                    for c in range(G):
                        e.matmul(ps[b][:, c, 0, :], S_ap, X[s][:, c, 0, :],
                                 start=True, stop=False).then_inc(s_pe, 1)
                        e.matmul(ps[b][:, c, 0, :], E_ap, X[s][:, c, 1, :],
                                 start=False, stop=True).then_inc(s_pe, 1)
                        e.matmul(ps[b][:, c, 1, :], S_ap, X[s][:, c, 1, :],
                                 start=True, stop=True).then_inc(s_pe, 1)

        @block.scalar
        def _(e):
            w = W(e)
            w(s_wv, wv_target)
            for gg in range(nGT):
                s = gg % NBUF
                b = gg % 2
                if pe_x1:
                    w(s_pe, 3 * G * (gg + 1))
                    src = ps[b]
                else:
                    w(s_ld[s], LD_PER_G * (gg // NBUF + 1))
                    src = X1[s]
                for ki in range(nK):
                    i = 4 * gg + ki
                    z = i % ZBUF
                    if i >= ZBUF:   # Z slot recycle: prior stores done
                        w(s_st[z], 32 * (i // ZBUF))
                    e.activation(Z[z][:, :, 0:1, :], src[:, :, 0:1, :], Copy,
                                 scale=wvt[:, col(1, ki, 0):col(1, ki, 0) + 1]
                                 ).then_inc(s_act, 1)
                    e.activation(Z[z][:, :, 1:2, :], src[:, :, 1:2, :], Copy,
                                 scale=wvt[:, col(1, ki, 1):col(1, ki, 1) + 1]
                                 ).then_inc(s_act, 1)

        @block.vector
        def _(e):
            w = W(e)
            e.memset(zeros[:, :, :, :], 0.0).then_inc(s_z, 1)
            w(s_wv, wv_target)
            for gg in range(nGT):
                s = gg % NBUF
                for ki in range(nK):
                    i = 4 * gg + ki
                    z = i % ZBUF
                    w(s_act, act_after(gg, ki, 1))
                    e.scalar_tensor_tensor(
                        Z[z][:, :, 0:1, :], X[s][:, :, 0:1, :],
                        wvt[:, col(0, ki, 0):col(0, ki, 0) + 1],
                        Z[z][:, :, 0:1, :], mult, add).then_inc(s_dve, 1)
                    w(s_act, act_after(gg, ki, 2))
                    e.scalar_tensor_tensor(
                        Z[z][:, :, 1:2, :], X[s][:, :, 1:2, :],
                        wvt[:, col(0, ki, 1):col(0, ki, 1) + 1],
                        Z[z][:, :, 1:2, :], mult, add).then_inc(s_dve, 1)

        @block.gpsimd
        def _(e):
            w = W(e)
            for gg in range(nGT):
                g = gg % nG
                for ki in range(nK):
                    k = ki + 1
                    d = int(dks[k])
                    i = 4 * gg + ki
                    z = i % ZBUF
                    og = out_h[k * C + g * G:k * C + (g + 1) * G, :, :]
                    w(s_st[z], 32 * (i // ZBUF))   # own-sem burst order
                    w(s_dve, 2 * i + 1)
                    e.dma_start(out=dram1(og[:, 0:H - d, :]),
                                in_=Z[z][d:H, :, 0:1, :]).then_inc(s_st[z], 16)
                    w(s_dve, 2 * i + 2)
                    e.dma_start(out=dram1(og[:, H - d:Fr - d, :]),
                                in_=Z[z][:, :, 1:2, :]).then_inc(s_st[z], 16)
    return nc


@functools.lru_cache(maxsize=1)
def _get_nc():
    return build_nc()


def _run(x, trace=False):
    in_maps = [{"x": np.ascontiguousarray(x[b])} for b in range(B)]
    res = run_bass_kernel_spmd(_get_nc(), in_maps, core_ids=list(range(N_CORES)),
                               trace=trace)
    out = np.stack([r["out"] for r in res.results], axis=0)
    return out, res


def kernel(x):
    x = np.asarray(x)
    assert x.shape == (B, C, F, T), x.shape
    out, _ = _run(x)
    return out
